# revision 37
# baseline (speedup 1.0000x reference)
"""Blended-expert MLP (MoE routing) Trainium2 Bass kernel.

Math: reference computes, per layer,
    h = elu( einsum("bi,bio->bo", x, einsum("be,eio->bio", c, w)) + c @ b )
which factorizes as
    h = elu( sum_e (c[:,e] * x) @ W_e  +  c @ b )
(row-scaling commutes with the matmul), so per layer we scale X^T by
c_e on the vector engine (8 ops) and run 8 [rows,512]x[512,512]
matmuls plus one tiny K=8 matmul for the blended bias, ALL accumulating
into a single PSUM tile. Then ELU, then a PE transpose to produce the
next layer's stationary operand.

Sharding: data-parallel over the batch. B=512 rows split across 8
NeuronCores (64 rows each); the expert weights are replicated to every
core (fp16: 12 MB/core, fully SBUF-resident). No collectives (on-chip
AllReduce has a ~20us latency floor, worse than replication).

Layout per core:
  stationary operand = (c_e * X)^T chunks [128(i), 64(b)]
  moving operand     = W chunks  [128(i), 4096(e,o)] sliced per expert
  psum out           = [128(2 expert-halves x 64b), 512(o)], fp32

fp16 operands (not bf16): same DMA bytes and same 1-cycle/row matmul
rate, but 10 mantissa bits give ~7e-4 relative error vs ~6e-3.

Performance structure (measured ~50-54us on hardware, best 49.6us):
  ~7.5us fixed NEFF preamble | ~33.5us weight-DMA window (the wall:
  12 MB at ~360 GB/s/core, fully overlapped with compute) | ~4us
  compute tail | ~6us fixed all-engine end barrier.
The weight window is the shared-HBM wall: all 8 cores on one chip pull
~408 GB/s each (~3.3 TB/s aggregate). A second HWDGE ring adds no
bandwidth in SPMD (verified), SWDGE int8->fp16 cast-DMA is write-side
bound (verified), PE has no int8 mode, and fp8 weights cost ~4.5%
error vs the 2% budget - so 12.58 MB fp16/core is the floor, and the
kernel is DMA-window-bound end to end.
Tricks that matter: PE warmup matmuls (HAM clock gate: cold PE runs at
1.2 GHz, warm 2.4 GHz), k-outer matmul order (each weight chunk's
matmuls fire on DMA arrival), even/odd expert pairs running
concurrently in the two column halves of the PE array (M=64 would
otherwise idle half the array), and a column-halved pipeline for the
psum-merge + ELU boundary chain.
"""

import numpy as np

B, E, D = 512, 8, 512
NCORES = 8
ROWS = B // NCORES  # 64
KC = D // 128  # 4 contraction chunks of 128

# pack tensor column layout (per 128 partitions)
PK_XT = 0  # [128, 256]: layer-1 x^T chunk k at cols [64k, 64k+64)
PK_CB = 256  # [128, 1024]: c broadcast; col 128e+64j+b = C[b,e], all partitions
PK_ID = PK_CB + E * 2 * ROWS  # [64, 64]: identity, partitions 0..63
PK_CT = PK_ID + ROWS  # [8, 64]: coef^T, partitions 0..7
PCK = PK_CT + ROWS

# matmul operand dtype: "f32" (exact, 4 cyc/row), "f32r" (fast fp32 mode,
# 1 cyc/row at N>=256), "bf16" (halves weight DMA, full-rate matmul).
MODE = "f16"

_NC_CACHE = {}


def _mmdt(mybir, mode):
    return {
        "f32": mybir.dt.float32,
        "f32r": mybir.dt.float32r,
        "bf16": mybir.dt.bfloat16,
        "f16": mybir.dt.float16,
    }[mode]


def _build(mode):
    from contextlib import ExitStack

    import concourse.bacc as bacc
    import concourse.mybir as mybir
    import concourse.tile as tile

    f32 = mybir.dt.float32
    mmdt = _mmdt(mybir, mode)
    Alu = mybir.AluOpType
    Act = mybir.ActivationFunctionType

    # Bacc (not raw Bass): its compile() legalizes the TRN2 one-sync-wait-
    # per-instruction limit by splitting excess waits into EventSemaphores
    nc = bacc.Bacc()
    pack_d = nc.declare_dram_parameter("pack", [128, PCK], mmdt, isOutput=False)
    bias_d = nc.declare_dram_parameter("biasd", [E, 3 * D], mmdt, isOutput=False)
    w_d = nc.declare_dram_parameter("w", [3, D, E * D], mmdt, isOutput=False)
    out_d = nc.declare_dram_parameter("out", [ROWS, D], f32, isOutput=True)

    with ExitStack() as ctx:
        tc = ctx.enter_context(tile.TileContext(nc))
        const = ctx.enter_context(tc.tile_pool(name="const", bufs=1))
        wpool = ctx.enter_context(
            tc.tile_pool(name="wp", bufs=12 if mode in ("bf16", "f16") else 8)
        )
        spool = ctx.enter_context(tc.tile_pool(name="sp", bufs=24))
        hpool = ctx.enter_context(tc.tile_pool(name="hp", bufs=2))
        xpool = ctx.enter_context(tc.tile_pool(name="xp", bufs=2))
        acc_ps = ctx.enter_context(tc.tile_pool(name="acc", bufs=3, space="PSUM"))
        pt_ps = ctx.enter_context(tc.tile_pool(name="pt", bufs=3, space="PSUM"))
        wm_ps = ctx.enter_context(tc.tile_pool(name="wm", bufs=1, space="PSUM"))

        # PE warmup: garbage matmuls on a zeroed tile (output never read),
        # emitted first so the HAM clock gate reaches 2.4 GHz before the
        # first real matmul (cold PE at 1.2 GHz otherwise doubles every
        # matmul). gpsimd is free right after its ~3.3us start preamble, so
        # it provides the earliest possible writer for the warm tile.
        # memset on DVE, not gpsimd: keeps gpsimd work out of the
        # pre-window start barrier, measured ~0.5us earlier first DMA
        warm = const.tile([128, ROWS + D], mmdt)
        nc.vector.memset(warm[:], 0.0)
        wps = wm_ps.tile([ROWS, D], f32, tag="warm")
        for _ in range(14):
            nc.tensor.matmul(
                wps[:], warm[:, 0:ROWS], warm[:, ROWS:], start=True, stop=True
            )

        pack_t = const.tile([128, PCK], mmdt)
        pack_dma = nc.sync.dma_start(pack_t[:], pack_d[:])
        # bias rides the scalar (qAct) HWDGE ring: its ~0.8us issue+transfer
        # on the sync ring would otherwise sit between pack and the first
        # weight chunk, delaying the whole HBM-bound weight window
        bias_t = const.tile([E, 3 * D], mmdt)
        nc.scalar.dma_start(bias_t[:], bias_d[:])

        coeft_ap = pack_t[0:E, PK_CT : PK_CT + ROWS]
        ident_ap = pack_t[0:ROWS, PK_ID : PK_ID + ROWS]
        xt_tile, xt_off = pack_t, PK_XT  # current x^T source: [128, 256] at offset

        # all weight-chunk DMAs up-front; the HWDGE lane round-robin plus
        # issue order paces them in consumption order at full bandwidth
        # (explicit chaining adds ~2us completion-latency per hop - worse)
        all_wts = []
        for layer in range(3):
            for k in range(KC):
                wt = wpool.tile([128, E * D], mmdt, tag="w")
                rs_ = slice(128 * k, 128 * (k + 1))
                if layer == 2 and k == KC - 1:
                    # split the final chunk: its first half (experts 0-3)
                    # lands ~1.2us earlier, so only experts 4-7's last
                    # matmuls gate on the very last 0.5 MB transfer
                    nc.sync.dma_start(
                        wt[:, 0 : E * D // 2], w_d[layer, rs_, 0 : E * D // 2]
                    )
                    nc.sync.dma_start(
                        wt[:, E * D // 2 :], w_d[layer, rs_, E * D // 2 :]
                    )
                else:
                    nc.sync.dma_start(wt[:], w_d[layer, rs_, :])
                all_wts.append(wt)

        for layer in range(3):
            wts = all_wts[layer * KC : (layer + 1) * KC]

            # scale x^T by c_e along the batch (free) dim: one DVE op per
            # expert over all 4 chunks at once
            # per-chunk rescale: TT(e,k) gates only on evacuation k of the
            # previous layer's transpose, and matmul (e,k) gates only on
            # TT(e,k) (subtile column tracking), so the boundary pipelines
            # at chunk granularity. c-broadcast is stored once (64 cols/e).
            # per chunk-pair rescale: TT(e,half) gates on the first/last two
            # transpose evacuations only, and matmul (e,k) gates on its half
            # (subtile column tracking)
            scaled = []
            for e in range(E):
                sc = spool.tile([128, KC * ROWS], mmdt, tag="sc")
                for half in range(2):
                    lo, hi = 2 * ROWS * half, 2 * ROWS * (half + 1)
                    nc.vector.tensor_tensor(
                        out=sc[:, lo:hi],
                        in0=xt_tile[:, xt_off + lo : xt_off + hi],
                        in1=pack_t[
                            :,
                            PK_CB + 2 * ROWS * e : PK_CB + 2 * ROWS * (e + 1),
                        ],
                        op=Alu.mult,
                    )
                scaled.append(sc)

            # one accumulation group: 32 expert matmuls + bias matmul (K=8).
            # k-outer order: each weight chunk's 8 expert matmuls fire as
            # soon as that chunk's DMA lands, overlapping the next transfer.
            # Experts 0-3 accumulate in psum half 0, 4-7 in half 1; pairs
            # (e, e+4) run CONCURRENTLY in the two column halves of the PE
            # array (tile_position), since M=64 only fills half the array.
            # The final chunk of layer 3 runs half-0 experts first (its DMA
            # is split the same way), so half 0 closes and its merge COPY
            # overlaps half 1's last matmuls.
            acc = acc_ps.tile([2 * ROWS, D], f32, tag="acc")
            # bias matmul opens the half-0 group
            nc.tensor.matmul(
                acc[0:ROWS, :],
                coeft_ap,
                bias_t[:, D * layer : D * (layer + 1)],
                start=True,
                stop=False,
                tile_position=(0, 0),
                skip_group_check=True,
            )
            EH = E // 2
            for k in range(KC):
                if layer == 2 and k == KC - 1:
                    order = list(range(E))  # 0-3 (half 0) then 4-7 (half 1)
                else:
                    order = [e for p in range(EH) for e in (p, p + EH)]
                for e in order:
                    half = e // EH
                    nc.tensor.matmul(
                        acc[half * ROWS : (half + 1) * ROWS, :],
                        scaled[e][:, ROWS * k : ROWS * (k + 1)],
                        wts[k][:, D * e : D * (e + 1)],
                        start=(k == 0 and e == EH),
                        stop=(k == KC - 1 and e in (EH - 1, E - 1)),
                        tile_position=(0, half * ROWS),
                        skip_group_check=True,
                    )
                if layer < 2:
                    # clock-sustain fillers: the ~0.7us DMA-wait gap after
                    # each chunk's matmuls would keep the HAM activity
                    # window below its busy threshold, leaving whole layers
                    # at 1.2 GHz on unlucky runs. Two ready-to-run warm
                    # matmuls per gap keep PE activity continuous; the
                    # mid-window PE slack absorbs their cost.
                    for _ in range(2):
                        nc.tensor.matmul(
                            wps[:], warm[:, 0:ROWS], warm[:, ROWS:],
                            start=True, stop=True,
                        )
            # evacuate even half (ACT) + merge halves (DVE) + elu + transpose,
            # pipelined per 128-column quarter: transpose k consumes exactly
            # quarter k, so each quarter flows through the whole boundary
            # chain independently
            t0 = hpool.tile([ROWS, D], f32, tag="t0")
            hpre = hpool.tile([ROWS, D], f32, tag="hpre")
            HD = D // 2
            if layer < 2:
                # keep the PE clock warm across the elu/transpose boundary
                # (a >3.4us PE-idle window would re-throttle to 1.2 GHz)
                for _ in range(8):
                    nc.tensor.matmul(
                        wps[:], warm[:, 0:ROWS], warm[:, ROWS:],
                        start=True, stop=True,
                    )

            if layer < 2:
                # per quarter q: copy+merge, elu(x)=max(x,0)+min(exp(x)-1,0),
                # then transpose + evacuation - all stages pipeline across
                # quarters on alternating engines
                ex = hpool.tile([ROWS, D], f32, tag="ex")
                h = hpool.tile([ROWS, D], mmdt, tag="h")
                xt_t = xpool.tile([128, KC * ROWS], mmdt, tag="xt")
                for q in range(KC):
                    qs = slice(128 * q, 128 * (q + 1))
                    nc.scalar.copy(t0[:, qs], acc[0:ROWS, qs])
                    nc.vector.tensor_tensor(
                        out=hpre[:, qs], in0=t0[:, qs], in1=acc[ROWS:, qs],
                        op=Alu.add,
                    )
                    nc.scalar.activation(ex[:, qs], hpre[:, qs], Act.Exp)
                    nc.vector.tensor_scalar(
                        ex[:, qs], ex[:, qs], 1.0, 0.0, Alu.subtract, Alu.min
                    )
                    nc.vector.scalar_tensor_tensor(
                        out=h[:, qs],
                        in0=hpre[:, qs],
                        scalar=0.0,
                        in1=ex[:, qs],
                        op0=Alu.max,
                        op1=Alu.add,
                    )
                    pt = pt_ps.tile([128, ROWS], mmdt, tag="pt")
                    nc.tensor.transpose(pt[:], h[:, qs], ident_ap)
                    dst = xt_t[:, ROWS * q : ROWS * (q + 1)]
                    if q % 2 == 0:
                        nc.scalar.copy(dst, pt[:])
                    else:
                        nc.vector.tensor_copy(dst, pt[:])
                xt_tile, xt_off = xt_t, 0
            else:
                # stream the output per column half, right behind the merge.
                # Each half gets its OWN t0/hpre tiles (tile-granular WAR
                # tracking otherwise serializes COPY(h1) behind ADD(h0)),
                # and both ACT copies are emitted before the DVE adds so
                # they run back-to-back, overlapping half 1's last matmuls.
                # The second out-DMA rides the scalar ring so the two issue
                # costs overlap.
                t0h = [
                    hpool.tile([ROWS, HD], f32, tag=f"ot{c}", name=f"t0h{c}")
                    for c in range(2)
                ]
                hpreh = [
                    hpool.tile([ROWS, HD], f32, tag=f"oh{c}", name=f"hpreh{c}")
                    for c in range(2)
                ]
                for c in range(2):
                    cs = slice(HD * c, HD * (c + 1))
                    nc.scalar.copy(t0h[c][:], acc[0:ROWS, cs])
                for c in range(2):
                    cs = slice(HD * c, HD * (c + 1))
                    nc.vector.tensor_tensor(
                        out=hpreh[c][:], in0=t0h[c][:], in1=acc[ROWS:, cs],
                        op=Alu.add,
                    )
                    eng = nc.sync if c == 0 else nc.scalar
                    eng.dma_start(out_d[:, cs], hpreh[c][:])

    nc.compile()
    return nc


def _get_nc(mode):
    if mode not in _NC_CACHE:
        _NC_CACHE[mode] = _build(mode)
    return _NC_CACHE[mode]


def _prep_in_maps(inputs, mode):
    import ml_dtypes

    X = np.asarray(inputs["X"], np.float32)
    C = np.asarray(inputs["blending_coef"], np.float32)
    ws = [np.asarray(inputs[f"w_l{i}"], np.float32) for i in (1, 2, 3)]
    bs = [np.asarray(inputs[f"b_l{i}"], np.float32) for i in (1, 2, 3)]

    mm_np = {
        "f32": np.float32,
        "f32r": np.float32,
        "bf16": ml_dtypes.bfloat16,
        "f16": np.float16,
    }[mode]

    # W[l][i, e*D+o] = w_l[e, i, o]
    W = np.stack([w.transpose(1, 0, 2).reshape(D, E * D) for w in ws]).astype(mm_np)
    Bb = np.concatenate(bs, axis=1).astype(mm_np)  # [E, 3*D]

    in_maps = []
    for c in range(NCORES):
        rs = slice(c * ROWS, (c + 1) * ROWS)
        pack = np.zeros((128, PCK), np.float32)
        # xt chunks: pack[p, 64k+b] = X[rows][b, 128k+p]
        xt = np.ascontiguousarray(X[rs].T)  # [512, 64]
        pack[:, PK_XT : PK_XT + KC * ROWS] = (
            xt.reshape(KC, 128, ROWS).transpose(1, 0, 2).reshape(128, KC * ROWS)
        )
        # c broadcast: pack[p, PK_CB + 128e + 64j + b] = C[rs][b, e]
        pack[:, PK_CB : PK_CB + E * 2 * ROWS] = np.broadcast_to(
            C[rs].T[:, None, :], (E, 2, ROWS)
        ).reshape(1, E * 2 * ROWS)
        pack[0:ROWS, PK_ID : PK_ID + ROWS] = np.eye(ROWS, dtype=np.float32)
        pack[0:E, PK_CT : PK_CT + ROWS] = C[rs].T
        in_maps.append({"pack": pack.astype(mm_np), "biasd": Bb, "w": W})
    return in_maps


def run(inputs, mode=MODE, trace=False):
    """Returns (output [512,512] fp32, BassKernelResults)."""
    from concourse.bass_utils import run_bass_kernel_spmd

    nc = _get_nc(mode)
    in_maps = _prep_in_maps(inputs, mode)
    res = run_bass_kernel_spmd(nc, in_maps, list(range(NCORES)), trace=trace)
    out = np.concatenate([r["out"] for r in res.results], axis=0)
    return out, res


def kernel(**inputs) -> np.ndarray:
    out, _ = run(inputs)
    return out


# revision 38
# speedup vs baseline: 1.0214x; 1.0214x over previous
"""Blended-expert MLP (MoE routing) Trainium2 Bass kernel.

Math: reference computes, per layer,
    h = elu( einsum("bi,bio->bo", x, einsum("be,eio->bio", c, w)) + c @ b )
which factorizes as
    h = elu( sum_e (c[:,e] * x) @ W_e  +  c @ b )
(row-scaling commutes with the matmul), so per layer we scale X^T by
c_e on the vector engine (8 ops) and run 8 [rows,512]x[512,512]
matmuls plus one tiny K=8 matmul for the blended bias, ALL accumulating
into a single PSUM tile. Then ELU, then a PE transpose to produce the
next layer's stationary operand.

Sharding: data-parallel over the batch. B=512 rows split across 8
NeuronCores (64 rows each); the expert weights are replicated to every
core (fp16: 12 MB/core, fully SBUF-resident). No collectives (on-chip
AllReduce has a ~20us latency floor, worse than replication).

Layout per core:
  stationary operand = (c_e * X)^T chunks [128(i), 64(b)]
  moving operand     = W chunks  [128(i), 4096(e,o)] sliced per expert
  psum out           = [128(2 expert-halves x 64b), 512(o)], fp32

fp16 operands (not bf16): same DMA bytes and same 1-cycle/row matmul
rate, but 10 mantissa bits give ~7e-4 relative error vs ~6e-3.

Performance structure (measured ~50-54us on hardware, best 49.6us):
  ~7.5us fixed NEFF preamble | ~33.5us weight-DMA window (the wall:
  12 MB at ~360 GB/s/core, fully overlapped with compute) | ~4us
  compute tail | ~6us fixed all-engine end barrier.
The weight window is the shared-HBM wall: all 8 cores on one chip pull
~408 GB/s each (~3.3 TB/s aggregate). A second HWDGE ring adds no
bandwidth in SPMD (verified), SWDGE int8->fp16 cast-DMA is write-side
bound (verified), PE has no int8 mode, and fp8 weights cost ~4.5%
error vs the 2% budget - so 12.58 MB fp16/core is the floor, and the
kernel is DMA-window-bound end to end.
Tricks that matter: PE warmup matmuls plus per-chunk clock-sustain
fillers (HAM clock gate: cold PE runs at 1.2 GHz, warm 2.4 GHz, and
the free-running activity window re-throttles whole layers on unlucky
runs), k-outer matmul order (each weight chunk's matmuls fire on DMA
arrival), expert pairs (e, e+4) running concurrently in the two column
halves of the PE array (M=64 would otherwise idle half the array) with
psum half 0 = experts 0-3 closing early on the final chunk, bias/pack
DMAs off the weight ring, and a column-halved pipeline with per-half
private tiles for the psum-merge + ELU boundary chain (tile-granular
dependency tracking otherwise serializes it).
"""

import numpy as np

B, E, D = 512, 8, 512
NCORES = 8
ROWS = B // NCORES  # 64
KC = D // 128  # 4 contraction chunks of 128

# pack tensor column layout (per 128 partitions). The c-broadcast table
# is NOT shipped from HBM: it is generated on-chip by a K=1 ones-matmul
# from a 2KB c-row (saves 256KB of the HBM-bound weight window).
PK_XT = 0  # [128, 256]: layer-1 x^T chunk k at cols [64k, 64k+64)
PK_ID = PK_XT + KC * ROWS  # [64, 64]: identity, partitions 0..63
PK_CT = PK_ID + ROWS  # [8, 64]: coef^T, partitions 0..7
PCK = PK_CT + ROWS
NCB = E * 2 * ROWS  # 1024: c-broadcast columns

# matmul operand dtype: "f32" (exact, 4 cyc/row), "f32r" (fast fp32 mode,
# 1 cyc/row at N>=256), "bf16" (halves weight DMA, full-rate matmul).
MODE = "f16"

_NC_CACHE = {}


def _mmdt(mybir, mode):
    return {
        "f32": mybir.dt.float32,
        "f32r": mybir.dt.float32r,
        "bf16": mybir.dt.bfloat16,
        "f16": mybir.dt.float16,
    }[mode]


def _build(mode):
    from contextlib import ExitStack

    import concourse.bacc as bacc
    import concourse.mybir as mybir
    import concourse.tile as tile

    f32 = mybir.dt.float32
    mmdt = _mmdt(mybir, mode)
    Alu = mybir.AluOpType
    Act = mybir.ActivationFunctionType

    # Bacc (not raw Bass): its compile() legalizes the TRN2 one-sync-wait-
    # per-instruction limit by splitting excess waits into EventSemaphores
    nc = bacc.Bacc()
    pack_d = nc.declare_dram_parameter("pack", [128, PCK], mmdt, isOutput=False)
    crow_d = nc.declare_dram_parameter("crow", [1, NCB], mmdt, isOutput=False)
    bias_d = nc.declare_dram_parameter("biasd", [E, 3 * D], mmdt, isOutput=False)
    w_d = nc.declare_dram_parameter("w", [3, D, E * D], mmdt, isOutput=False)
    out_d = nc.declare_dram_parameter("out", [ROWS, D], f32, isOutput=True)

    with ExitStack() as ctx:
        tc = ctx.enter_context(tile.TileContext(nc))
        const = ctx.enter_context(tc.tile_pool(name="const", bufs=1))
        wpool = ctx.enter_context(
            tc.tile_pool(name="wp", bufs=12 if mode in ("bf16", "f16") else 8)
        )
        spool = ctx.enter_context(tc.tile_pool(name="sp", bufs=24))
        hpool = ctx.enter_context(tc.tile_pool(name="hp", bufs=2))
        xpool = ctx.enter_context(tc.tile_pool(name="xp", bufs=2))
        acc_ps = ctx.enter_context(tc.tile_pool(name="acc", bufs=3, space="PSUM"))
        pt_ps = ctx.enter_context(tc.tile_pool(name="pt", bufs=3, space="PSUM"))
        wm_ps = ctx.enter_context(tc.tile_pool(name="wm", bufs=1, space="PSUM"))

        # PE warmup: garbage matmuls on a zeroed tile (output never read),
        # emitted first so the HAM clock gate reaches 2.4 GHz before the
        # first real matmul (cold PE at 1.2 GHz otherwise doubles every
        # matmul). gpsimd is free right after its ~3.3us start preamble, so
        # it provides the earliest possible writer for the warm tile.
        # memset on DVE, not gpsimd: keeps gpsimd work out of the
        # pre-window start barrier, measured ~0.5us earlier first DMA
        warm = const.tile([128, ROWS + D], mmdt)
        nc.vector.memset(warm[:], 0.0)
        ones_t = const.tile([1, 128], mmdt)
        nc.vector.memset(ones_t[:], 1.0)
        wps = wm_ps.tile([ROWS, D], f32, tag="warm")
        for _ in range(3):
            nc.tensor.matmul(
                wps[:], warm[:, 0:ROWS], warm[:, ROWS:], start=True, stop=True
            )

        # on-chip c-broadcast: cb[p, 128e+64j+b] = C[b,e] for all p, built
        # by ones[1,128].T @ crow[1,512-half] (K=1 matmul) into PSUM, then
        # an ACT copy down to fp16 SBUF. Replaces a 256KB HBM table with a
        # 2KB row + two early PE matmuls (which double as clock warmup).
        # crow at the HEAD of the sync ring: 2KB costs nothing there, and
        # the scalar ring's ACT-table-load would delay it ~2us past the
        # bc matmuls' slot in the PE ramp
        crow_t = const.tile([1, NCB], mmdt)
        nc.sync.dma_start(crow_t[:], crow_d[:])
        cb_t = const.tile([128, NCB], mmdt)
        for h2 in range(2):
            lo, hi = (NCB // 2) * h2, (NCB // 2) * (h2 + 1)
            bc = wm_ps.tile([128, NCB // 2], f32, tag="bc")
            nc.tensor.matmul(
                bc[:], ones_t[0:1, :], crow_t[0:1, lo:hi], start=True, stop=True
            )
            nc.scalar.copy(cb_t[:, lo:hi], bc[:])
        for _ in range(11):
            nc.tensor.matmul(
                wps[:], warm[:, 0:ROWS], warm[:, ROWS:], start=True, stop=True
            )

        pack_t = const.tile([128, PCK], mmdt)
        nc.sync.dma_start(pack_t[:], pack_d[:])
        # bias rides the scalar (qAct) HWDGE ring: its ~0.8us issue+transfer
        # on the sync ring would otherwise sit between pack and the first
        # weight chunk, delaying the whole HBM-bound weight window
        bias_t = const.tile([E, 3 * D], mmdt)
        nc.scalar.dma_start(bias_t[:], bias_d[:])

        coeft_ap = pack_t[0:E, PK_CT : PK_CT + ROWS]
        ident_ap = pack_t[0:ROWS, PK_ID : PK_ID + ROWS]
        xt_tile, xt_off = pack_t, PK_XT  # current x^T source: [128, 256] at offset

        # all weight-chunk DMAs up-front; the HWDGE lane round-robin plus
        # issue order paces them in consumption order at full bandwidth
        # (explicit chaining adds ~2us completion-latency per hop - worse)
        all_wts = []
        for layer in range(3):
            for k in range(KC):
                wt = wpool.tile([128, E * D], mmdt, tag="w")
                rs_ = slice(128 * k, 128 * (k + 1))
                if layer == 2 and k == KC - 1:
                    # split the final chunk: its first half (experts 0-3)
                    # lands ~1.2us earlier, so only experts 4-7's last
                    # matmuls gate on the very last 0.5 MB transfer
                    nc.sync.dma_start(
                        wt[:, 0 : E * D // 2], w_d[layer, rs_, 0 : E * D // 2]
                    )
                    nc.sync.dma_start(
                        wt[:, E * D // 2 :], w_d[layer, rs_, E * D // 2 :]
                    )
                else:
                    nc.sync.dma_start(wt[:], w_d[layer, rs_, :])
                all_wts.append(wt)

        for layer in range(3):
            wts = all_wts[layer * KC : (layer + 1) * KC]

            # scale x^T by c_e along the batch (free) dim: one DVE op per
            # expert over all 4 chunks at once
            # per-chunk rescale: TT(e,k) gates only on evacuation k of the
            # previous layer's transpose, and matmul (e,k) gates only on
            # TT(e,k) (subtile column tracking), so the boundary pipelines
            # at chunk granularity. c-broadcast is stored once (64 cols/e).
            # per chunk-pair rescale: TT(e,half) gates on the first/last two
            # transpose evacuations only, and matmul (e,k) gates on its half
            # (subtile column tracking)
            scaled = []
            for e in range(E):
                sc = spool.tile([128, KC * ROWS], mmdt, tag="sc")
                for half in range(2):
                    lo, hi = 2 * ROWS * half, 2 * ROWS * (half + 1)
                    nc.vector.tensor_tensor(
                        out=sc[:, lo:hi],
                        in0=xt_tile[:, xt_off + lo : xt_off + hi],
                        in1=cb_t[:, 2 * ROWS * e : 2 * ROWS * (e + 1)],
                        op=Alu.mult,
                    )
                scaled.append(sc)

            # one accumulation group: 32 expert matmuls + bias matmul (K=8).
            # k-outer order: each weight chunk's 8 expert matmuls fire as
            # soon as that chunk's DMA lands, overlapping the next transfer.
            # Experts 0-3 accumulate in psum half 0, 4-7 in half 1; pairs
            # (e, e+4) run CONCURRENTLY in the two column halves of the PE
            # array (tile_position), since M=64 only fills half the array.
            # The final chunk of layer 3 runs half-0 experts first (its DMA
            # is split the same way), so half 0 closes and its merge COPY
            # overlaps half 1's last matmuls.
            acc = acc_ps.tile([2 * ROWS, D], f32, tag="acc")
            # bias matmul opens the half-0 group
            nc.tensor.matmul(
                acc[0:ROWS, :],
                coeft_ap,
                bias_t[:, D * layer : D * (layer + 1)],
                start=True,
                stop=False,
                tile_position=(0, 0),
                skip_group_check=True,
            )
            EH = E // 2
            for k in range(KC):
                if layer == 2 and k == KC - 1:
                    order = list(range(E))  # 0-3 (half 0) then 4-7 (half 1)
                else:
                    order = [e for p in range(EH) for e in (p, p + EH)]
                for e in order:
                    half = e // EH
                    nc.tensor.matmul(
                        acc[half * ROWS : (half + 1) * ROWS, :],
                        scaled[e][:, ROWS * k : ROWS * (k + 1)],
                        wts[k][:, D * e : D * (e + 1)],
                        start=(k == 0 and e == EH),
                        stop=(k == KC - 1 and e in (EH - 1, E - 1)),
                        tile_position=(0, half * ROWS),
                        skip_group_check=True,
                    )
                if layer < 2:
                    # clock-sustain fillers: the ~0.7us DMA-wait gap after
                    # each chunk's matmuls would keep the HAM activity
                    # window below its busy threshold, leaving whole layers
                    # at 1.2 GHz on unlucky runs. Two ready-to-run warm
                    # matmuls per gap keep PE activity continuous; the
                    # mid-window PE slack absorbs their cost.
                    for _ in range(2):
                        nc.tensor.matmul(
                            wps[:], warm[:, 0:ROWS], warm[:, ROWS:],
                            start=True, stop=True,
                        )
            # evacuate even half (ACT) + merge halves (DVE) + elu + transpose,
            # pipelined per 128-column quarter: transpose k consumes exactly
            # quarter k, so each quarter flows through the whole boundary
            # chain independently
            t0 = hpool.tile([ROWS, D], f32, tag="t0")
            hpre = hpool.tile([ROWS, D], f32, tag="hpre")
            HD = D // 2
            if layer < 2:
                # per quarter q: copy+merge, elu(x)=max(x,0)+min(exp(x)-1,0),
                # then transpose + evacuation - all stages pipeline across
                # quarters on alternating engines
                ex = hpool.tile([ROWS, D], f32, tag="ex")
                h = hpool.tile([ROWS, D], mmdt, tag="h")
                xt_t = xpool.tile([128, KC * ROWS], mmdt, tag="xt")
                for q in range(KC):
                    qs = slice(128 * q, 128 * (q + 1))
                    nc.scalar.copy(t0[:, qs], acc[0:ROWS, qs])
                    nc.vector.tensor_tensor(
                        out=hpre[:, qs], in0=t0[:, qs], in1=acc[ROWS:, qs],
                        op=Alu.add,
                    )
                    nc.scalar.activation(ex[:, qs], hpre[:, qs], Act.Exp)
                    nc.vector.tensor_scalar(
                        ex[:, qs], ex[:, qs], 1.0, 0.0, Alu.subtract, Alu.min
                    )
                    nc.vector.scalar_tensor_tensor(
                        out=h[:, qs],
                        in0=hpre[:, qs],
                        scalar=0.0,
                        in1=ex[:, qs],
                        op0=Alu.max,
                        op1=Alu.add,
                    )
                    # 2 ready-to-run keepwarms ahead of each transpose keep
                    # PE activity continuous across the boundary (the HAM
                    # clock gate re-throttles whole layers on unlucky runs
                    # if the activity window goes quiet here)
                    for _ in range(2):
                        nc.tensor.matmul(
                            wps[:], warm[:, 0:ROWS], warm[:, ROWS:],
                            start=True, stop=True,
                        )
                    pt = pt_ps.tile([128, ROWS], mmdt, tag="pt")
                    nc.tensor.transpose(pt[:], h[:, qs], ident_ap)
                    dst = xt_t[:, ROWS * q : ROWS * (q + 1)]
                    if q % 2 == 0:
                        nc.scalar.copy(dst, pt[:])
                    else:
                        nc.vector.tensor_copy(dst, pt[:])
                xt_tile, xt_off = xt_t, 0
            else:
                # stream the output per column half, right behind the merge.
                # Each half gets its OWN t0/hpre tiles (tile-granular WAR
                # tracking otherwise serializes COPY(h1) behind ADD(h0)),
                # and both ACT copies are emitted before the DVE adds so
                # they run back-to-back, overlapping half 1's last matmuls.
                # The second out-DMA rides the scalar ring so the two issue
                # costs overlap.
                t0h = [
                    hpool.tile([ROWS, HD], f32, tag=f"ot{c}", name=f"t0h{c}")
                    for c in range(2)
                ]
                hpreh = [
                    hpool.tile([ROWS, HD], f32, tag=f"oh{c}", name=f"hpreh{c}")
                    for c in range(2)
                ]
                for c in range(2):
                    cs = slice(HD * c, HD * (c + 1))
                    nc.scalar.copy(t0h[c][:], acc[0:ROWS, cs])
                for c in range(2):
                    cs = slice(HD * c, HD * (c + 1))
                    nc.vector.tensor_tensor(
                        out=hpreh[c][:], in0=t0h[c][:], in1=acc[ROWS:, cs],
                        op=Alu.add,
                    )
                    eng = nc.sync if c == 0 else nc.scalar
                    eng.dma_start(out_d[:, cs], hpreh[c][:])

    nc.compile()
    return nc


def _get_nc(mode):
    if mode not in _NC_CACHE:
        _NC_CACHE[mode] = _build(mode)
    return _NC_CACHE[mode]


def _prep_in_maps(inputs, mode):
    import ml_dtypes

    X = np.asarray(inputs["X"], np.float32)
    C = np.asarray(inputs["blending_coef"], np.float32)
    ws = [np.asarray(inputs[f"w_l{i}"], np.float32) for i in (1, 2, 3)]
    bs = [np.asarray(inputs[f"b_l{i}"], np.float32) for i in (1, 2, 3)]

    mm_np = {
        "f32": np.float32,
        "f32r": np.float32,
        "bf16": ml_dtypes.bfloat16,
        "f16": np.float16,
    }[mode]

    # W[l][i, e*D+o] = w_l[e, i, o]
    W = np.stack([w.transpose(1, 0, 2).reshape(D, E * D) for w in ws]).astype(mm_np)
    Bb = np.concatenate(bs, axis=1).astype(mm_np)  # [E, 3*D]

    in_maps = []
    for c in range(NCORES):
        rs = slice(c * ROWS, (c + 1) * ROWS)
        pack = np.zeros((128, PCK), np.float32)
        # xt chunks: pack[p, 64k+b] = X[rows][b, 128k+p]
        xt = np.ascontiguousarray(X[rs].T)  # [512, 64]
        pack[:, PK_XT : PK_XT + KC * ROWS] = (
            xt.reshape(KC, 128, ROWS).transpose(1, 0, 2).reshape(128, KC * ROWS)
        )
        pack[0:ROWS, PK_ID : PK_ID + ROWS] = np.eye(ROWS, dtype=np.float32)
        pack[0:E, PK_CT : PK_CT + ROWS] = C[rs].T
        # c-broadcast row (expanded to all 128 partitions on-chip):
        # crow[0, 128e + 64j + b] = C[rs][b, e]
        crow = np.broadcast_to(C[rs].T[:, None, :], (E, 2, ROWS)).reshape(1, -1)
        in_maps.append(
            {
                "pack": pack.astype(mm_np),
                "crow": crow.astype(mm_np),
                "biasd": Bb,
                "w": W,
            }
        )
    return in_maps


def run(inputs, mode=MODE, trace=False):
    """Returns (output [512,512] fp32, BassKernelResults)."""
    from concourse.bass_utils import run_bass_kernel_spmd

    nc = _get_nc(mode)
    in_maps = _prep_in_maps(inputs, mode)
    res = run_bass_kernel_spmd(nc, in_maps, list(range(NCORES)), trace=trace)
    out = np.concatenate([r["out"] for r in res.results], axis=0)
    return out, res


def kernel(**inputs) -> np.ndarray:
    out, _ = run(inputs)
    return out


# revision 39
# speedup vs baseline: 1.0216x; 1.0002x over previous
"""Blended-expert MLP (MoE routing) Trainium2 Bass kernel.

Math: reference computes, per layer,
    h = elu( einsum("bi,bio->bo", x, einsum("be,eio->bio", c, w)) + c @ b )
which factorizes as
    h = elu( sum_e (c[:,e] * x) @ W_e  +  c @ b )
(row-scaling commutes with the matmul), so per layer we scale X^T by
c_e on the vector engine (8 ops) and run 8 [rows,512]x[512,512]
matmuls plus one tiny K=8 matmul for the blended bias, ALL accumulating
into a single PSUM tile. Then ELU, then a PE transpose to produce the
next layer's stationary operand.

Sharding: data-parallel over the batch. B=512 rows split across 8
NeuronCores (64 rows each); the expert weights are replicated to every
core (fp16: 12 MB/core, fully SBUF-resident). No collectives (on-chip
AllReduce has a ~20us latency floor, worse than replication).

Layout per core:
  stationary operand = (c_e * X)^T chunks [128(i), 64(b)]
  moving operand     = W chunks  [128(i), 4096(e,o)] sliced per expert
  psum out           = [128(2 expert-halves x 64b), 512(o)], fp32

fp16 operands (not bf16): same DMA bytes and same 1-cycle/row matmul
rate, but 10 mantissa bits give ~7e-4 relative error vs ~6e-3.

Performance structure (measured ~50-54us on hardware, best 49.6us):
  ~7.5us fixed NEFF preamble | ~33.5us weight-DMA window (the wall:
  12 MB at ~360 GB/s/core, fully overlapped with compute) | ~4us
  compute tail | ~6us fixed all-engine end barrier.
The weight window is the shared-HBM wall: all 8 cores on one chip pull
~408 GB/s each (~3.3 TB/s aggregate). A second HWDGE ring adds no
bandwidth in SPMD (verified), SWDGE int8->fp16 cast-DMA is write-side
bound (verified), PE has no int8 mode, and fp8 weights cost ~4.5%
error vs the 2% budget - so 12.58 MB fp16/core is the floor, and the
kernel is DMA-window-bound end to end.
Tricks that matter: PE warmup matmuls plus per-chunk clock-sustain
fillers (HAM clock gate: cold PE runs at 1.2 GHz, warm 2.4 GHz, and
the free-running activity window re-throttles whole layers on unlucky
runs), k-outer matmul order (each weight chunk's matmuls fire on DMA
arrival), expert pairs (e, e+4) running concurrently in the two column
halves of the PE array (M=64 would otherwise idle half the array) with
psum half 0 = experts 0-3 closing early on the final chunk, bias/pack
DMAs off the weight ring, and a column-halved pipeline with per-half
private tiles for the psum-merge + ELU boundary chain (tile-granular
dependency tracking otherwise serializes it).
"""

import numpy as np

B, E, D = 512, 8, 512
NCORES = 8
ROWS = B // NCORES  # 64
KC = D // 128  # 4 contraction chunks of 128

# pack tensor column layout (per 128 partitions). The c-broadcast table
# is NOT shipped from HBM: it is generated on-chip by a K=1 ones-matmul
# from a 2KB c-row (saves 256KB of the HBM-bound weight window).
PK_XT = 0  # [128, 256]: layer-1 x^T chunk k at cols [64k, 64k+64)
PK_ID = PK_XT + KC * ROWS  # [64, 64]: identity, partitions 0..63
PK_CT = PK_ID + ROWS  # [8, 64]: coef^T, partitions 0..7
PCK = PK_CT + ROWS
NCB = E * 2 * ROWS  # 1024: c-broadcast columns

# matmul operand dtype: "f32" (exact, 4 cyc/row), "f32r" (fast fp32 mode,
# 1 cyc/row at N>=256), "bf16" (halves weight DMA, full-rate matmul).
MODE = "f16"

_NC_CACHE = {}


def _mmdt(mybir, mode):
    return {
        "f32": mybir.dt.float32,
        "f32r": mybir.dt.float32r,
        "bf16": mybir.dt.bfloat16,
        "f16": mybir.dt.float16,
    }[mode]


def _build(mode):
    from contextlib import ExitStack

    import concourse.bacc as bacc
    import concourse.mybir as mybir
    import concourse.tile as tile

    f32 = mybir.dt.float32
    mmdt = _mmdt(mybir, mode)
    Alu = mybir.AluOpType
    Act = mybir.ActivationFunctionType

    # Bacc (not raw Bass): its compile() legalizes the TRN2 one-sync-wait-
    # per-instruction limit by splitting excess waits into EventSemaphores
    nc = bacc.Bacc()
    pack_d = nc.declare_dram_parameter("pack", [128, PCK], mmdt, isOutput=False)
    crow_d = nc.declare_dram_parameter("crow", [1, NCB], mmdt, isOutput=False)
    bias_d = nc.declare_dram_parameter("biasd", [E, 3 * D], mmdt, isOutput=False)
    w_d = nc.declare_dram_parameter("w", [3, D, E * D], mmdt, isOutput=False)
    out_d = nc.declare_dram_parameter("out", [ROWS, D], f32, isOutput=True)

    with ExitStack() as ctx:
        tc = ctx.enter_context(tile.TileContext(nc))
        const = ctx.enter_context(tc.tile_pool(name="const", bufs=1))
        wpool = ctx.enter_context(
            tc.tile_pool(name="wp", bufs=12 if mode in ("bf16", "f16") else 8)
        )
        spool = ctx.enter_context(tc.tile_pool(name="sp", bufs=24))
        hpool = ctx.enter_context(tc.tile_pool(name="hp", bufs=2))
        xpool = ctx.enter_context(tc.tile_pool(name="xp", bufs=2))
        acc_ps = ctx.enter_context(tc.tile_pool(name="acc", bufs=3, space="PSUM"))
        pt_ps = ctx.enter_context(tc.tile_pool(name="pt", bufs=3, space="PSUM"))
        wm_ps = ctx.enter_context(tc.tile_pool(name="wm", bufs=1, space="PSUM"))

        # PE warmup: garbage matmuls on a zeroed tile (output never read),
        # emitted first so the HAM clock gate reaches 2.4 GHz before the
        # first real matmul (cold PE at 1.2 GHz otherwise doubles every
        # matmul). gpsimd is free right after its ~3.3us start preamble, so
        # it provides the earliest possible writer for the warm tile.
        # memset on DVE, not gpsimd: keeps gpsimd work out of the
        # pre-window start barrier, measured ~0.5us earlier first DMA
        warm = const.tile([128, ROWS + D], mmdt)
        nc.vector.memset(warm[:], 0.0)
        ones_t = const.tile([1, 128], mmdt)
        nc.vector.memset(ones_t[:], 1.0)
        wps = wm_ps.tile([ROWS, D], f32, tag="warm")
        for _ in range(3):
            nc.tensor.matmul(
                wps[:], warm[:, 0:ROWS], warm[:, ROWS:], start=True, stop=True
            )

        # on-chip c-broadcast: cb[p, 128e+64j+b] = C[b,e] for all p, built
        # by ones[1,128].T @ crow[1,512-half] (K=1 matmul) into PSUM, then
        # an ACT copy down to fp16 SBUF. Replaces a 256KB HBM table with a
        # 2KB row + two early PE matmuls (which double as clock warmup).
        # crow at the HEAD of the sync ring: 2KB costs nothing there, and
        # the scalar ring's ACT-table-load would delay it ~2us past the
        # bc matmuls' slot in the PE ramp
        crow_t = const.tile([1, NCB], mmdt)
        nc.sync.dma_start(crow_t[:], crow_d[:])
        cb_t = const.tile([128, NCB], mmdt)
        for h2 in range(2):
            lo, hi = (NCB // 2) * h2, (NCB // 2) * (h2 + 1)
            bc = wm_ps.tile([128, NCB // 2], f32, tag="bc")
            nc.tensor.matmul(
                bc[:], ones_t[0:1, :], crow_t[0:1, lo:hi], start=True, stop=True
            )
            nc.scalar.copy(cb_t[:, lo:hi], bc[:])
        for _ in range(11):
            nc.tensor.matmul(
                wps[:], warm[:, 0:ROWS], warm[:, ROWS:], start=True, stop=True
            )

        pack_t = const.tile([128, PCK], mmdt)
        nc.sync.dma_start(pack_t[:], pack_d[:])
        # bias rides the scalar (qAct) HWDGE ring: its ~0.8us issue+transfer
        # on the sync ring would otherwise sit between pack and the first
        # weight chunk, delaying the whole HBM-bound weight window
        bias_t = const.tile([E, 3 * D], mmdt)
        nc.scalar.dma_start(bias_t[:], bias_d[:])

        coeft_ap = pack_t[0:E, PK_CT : PK_CT + ROWS]
        ident_ap = pack_t[0:ROWS, PK_ID : PK_ID + ROWS]
        xt_tile, xt_off = pack_t, PK_XT  # current x^T source: [128, 256] at offset

        # all weight-chunk DMAs up-front; the HWDGE lane round-robin plus
        # issue order paces them in consumption order at full bandwidth
        # (explicit chaining adds ~2us completion-latency per hop - worse)
        all_wts = []
        for layer in range(3):
            for k in range(KC):
                wt = wpool.tile([128, E * D], mmdt, tag="w")
                rs_ = slice(128 * k, 128 * (k + 1))
                if layer == 2 and k == KC - 1:
                    # split the final chunk into 4 expert-pair pieces: each
                    # piece's completion sem fires as it lands, so the k=3
                    # matmuls (serial order e=0..7) start progressively and
                    # only experts 6-7 gate on the very last 0.25 MB piece
                    # plus its ~1-2us completion-receipt latency
                    q4 = E * D // 4
                    for piece in range(4):
                        nc.sync.dma_start(
                            wt[:, piece * q4 : (piece + 1) * q4],
                            w_d[layer, rs_, piece * q4 : (piece + 1) * q4],
                        )
                else:
                    nc.sync.dma_start(wt[:], w_d[layer, rs_, :])
                all_wts.append(wt)

        for layer in range(3):
            wts = all_wts[layer * KC : (layer + 1) * KC]

            # scale x^T by c_e along the batch (free) dim: one DVE op per
            # expert over all 4 chunks at once
            # per-chunk rescale: TT(e,k) gates only on evacuation k of the
            # previous layer's transpose, and matmul (e,k) gates only on
            # TT(e,k) (subtile column tracking), so the boundary pipelines
            # at chunk granularity. c-broadcast is stored once (64 cols/e).
            # per chunk-pair rescale: TT(e,half) gates on the first/last two
            # transpose evacuations only, and matmul (e,k) gates on its half
            # (subtile column tracking)
            scaled = []
            for e in range(E):
                sc = spool.tile([128, KC * ROWS], mmdt, tag="sc")
                for half in range(2):
                    lo, hi = 2 * ROWS * half, 2 * ROWS * (half + 1)
                    nc.vector.tensor_tensor(
                        out=sc[:, lo:hi],
                        in0=xt_tile[:, xt_off + lo : xt_off + hi],
                        in1=cb_t[:, 2 * ROWS * e : 2 * ROWS * (e + 1)],
                        op=Alu.mult,
                    )
                scaled.append(sc)

            # one accumulation group: 32 expert matmuls + bias matmul (K=8).
            # k-outer order: each weight chunk's 8 expert matmuls fire as
            # soon as that chunk's DMA lands, overlapping the next transfer.
            # Experts 0-3 accumulate in psum half 0, 4-7 in half 1; pairs
            # (e, e+4) run CONCURRENTLY in the two column halves of the PE
            # array (tile_position), since M=64 only fills half the array.
            # The final chunk of layer 3 runs half-0 experts first (its DMA
            # is split the same way), so half 0 closes and its merge COPY
            # overlaps half 1's last matmuls.
            acc = acc_ps.tile([2 * ROWS, D], f32, tag="acc")
            # bias matmul opens the half-0 group
            nc.tensor.matmul(
                acc[0:ROWS, :],
                coeft_ap,
                bias_t[:, D * layer : D * (layer + 1)],
                start=True,
                stop=False,
                tile_position=(0, 0),
                skip_group_check=True,
            )
            EH = E // 2
            for k in range(KC):
                if layer == 2 and k == KC - 1:
                    order = list(range(E))  # 0-3 (half 0) then 4-7 (half 1)
                else:
                    order = [e for p in range(EH) for e in (p, p + EH)]
                for e in order:
                    half = e // EH
                    nc.tensor.matmul(
                        acc[half * ROWS : (half + 1) * ROWS, :],
                        scaled[e][:, ROWS * k : ROWS * (k + 1)],
                        wts[k][:, D * e : D * (e + 1)],
                        start=(k == 0 and e == EH),
                        stop=(k == KC - 1 and e in (EH - 1, E - 1)),
                        tile_position=(0, half * ROWS),
                        skip_group_check=True,
                    )
                if layer < 2:
                    # clock-sustain fillers: the ~0.7us DMA-wait gap after
                    # each chunk's matmuls would keep the HAM activity
                    # window below its busy threshold, leaving whole layers
                    # at 1.2 GHz on unlucky runs. Two ready-to-run warm
                    # matmuls per gap keep PE activity continuous; the
                    # mid-window PE slack absorbs their cost.
                    for _ in range(2):
                        nc.tensor.matmul(
                            wps[:], warm[:, 0:ROWS], warm[:, ROWS:],
                            start=True, stop=True,
                        )
            # evacuate even half (ACT) + merge halves (DVE) + elu + transpose,
            # pipelined per 128-column quarter: transpose k consumes exactly
            # quarter k, so each quarter flows through the whole boundary
            # chain independently
            t0 = hpool.tile([ROWS, D], f32, tag="t0")
            hpre = hpool.tile([ROWS, D], f32, tag="hpre")
            HD = D // 2
            if layer < 2:
                # per quarter q: copy+merge, elu(x)=max(x,0)+min(exp(x)-1,0),
                # then transpose + evacuation - all stages pipeline across
                # quarters on alternating engines
                ex = hpool.tile([ROWS, D], f32, tag="ex")
                h = hpool.tile([ROWS, D], mmdt, tag="h")
                xt_t = xpool.tile([128, KC * ROWS], mmdt, tag="xt")
                for q in range(KC):
                    qs = slice(128 * q, 128 * (q + 1))
                    nc.scalar.copy(t0[:, qs], acc[0:ROWS, qs])
                    nc.vector.tensor_tensor(
                        out=hpre[:, qs], in0=t0[:, qs], in1=acc[ROWS:, qs],
                        op=Alu.add,
                    )
                    nc.scalar.activation(ex[:, qs], hpre[:, qs], Act.Exp)
                    nc.vector.tensor_scalar(
                        ex[:, qs], ex[:, qs], 1.0, 0.0, Alu.subtract, Alu.min
                    )
                    nc.vector.scalar_tensor_tensor(
                        out=h[:, qs],
                        in0=hpre[:, qs],
                        scalar=0.0,
                        in1=ex[:, qs],
                        op0=Alu.max,
                        op1=Alu.add,
                    )
                    # 2 ready-to-run keepwarms ahead of each transpose keep
                    # PE activity continuous across the boundary (the HAM
                    # clock gate re-throttles whole layers on unlucky runs
                    # if the activity window goes quiet here)
                    for _ in range(2):
                        nc.tensor.matmul(
                            wps[:], warm[:, 0:ROWS], warm[:, ROWS:],
                            start=True, stop=True,
                        )
                    pt = pt_ps.tile([128, ROWS], mmdt, tag="pt")
                    nc.tensor.transpose(pt[:], h[:, qs], ident_ap)
                    dst = xt_t[:, ROWS * q : ROWS * (q + 1)]
                    if q % 2 == 0:
                        nc.scalar.copy(dst, pt[:])
                    else:
                        nc.vector.tensor_copy(dst, pt[:])
                xt_tile, xt_off = xt_t, 0
            else:
                # stream the output per column half, right behind the merge.
                # Each half gets its OWN t0/hpre tiles (tile-granular WAR
                # tracking otherwise serializes COPY(h1) behind ADD(h0)),
                # and both ACT copies are emitted before the DVE adds so
                # they run back-to-back, overlapping half 1's last matmuls.
                # The second out-DMA rides the scalar ring so the two issue
                # costs overlap.
                t0h = [
                    hpool.tile([ROWS, HD], f32, tag=f"ot{c}", name=f"t0h{c}")
                    for c in range(2)
                ]
                hpreh = [
                    hpool.tile([ROWS, HD], f32, tag=f"oh{c}", name=f"hpreh{c}")
                    for c in range(2)
                ]
                for c in range(2):
                    cs = slice(HD * c, HD * (c + 1))
                    nc.scalar.copy(t0h[c][:], acc[0:ROWS, cs])
                for c in range(2):
                    cs = slice(HD * c, HD * (c + 1))
                    nc.vector.tensor_tensor(
                        out=hpreh[c][:], in0=t0h[c][:], in1=acc[ROWS:, cs],
                        op=Alu.add,
                    )
                    eng = nc.sync if c == 0 else nc.scalar
                    eng.dma_start(out_d[:, cs], hpreh[c][:])

    nc.compile()
    return nc


def _get_nc(mode):
    if mode not in _NC_CACHE:
        _NC_CACHE[mode] = _build(mode)
    return _NC_CACHE[mode]


def _prep_in_maps(inputs, mode):
    import ml_dtypes

    X = np.asarray(inputs["X"], np.float32)
    C = np.asarray(inputs["blending_coef"], np.float32)
    ws = [np.asarray(inputs[f"w_l{i}"], np.float32) for i in (1, 2, 3)]
    bs = [np.asarray(inputs[f"b_l{i}"], np.float32) for i in (1, 2, 3)]

    mm_np = {
        "f32": np.float32,
        "f32r": np.float32,
        "bf16": ml_dtypes.bfloat16,
        "f16": np.float16,
    }[mode]

    # W[l][i, e*D+o] = w_l[e, i, o]
    W = np.stack([w.transpose(1, 0, 2).reshape(D, E * D) for w in ws]).astype(mm_np)
    Bb = np.concatenate(bs, axis=1).astype(mm_np)  # [E, 3*D]

    in_maps = []
    for c in range(NCORES):
        rs = slice(c * ROWS, (c + 1) * ROWS)
        pack = np.zeros((128, PCK), np.float32)
        # xt chunks: pack[p, 64k+b] = X[rows][b, 128k+p]
        xt = np.ascontiguousarray(X[rs].T)  # [512, 64]
        pack[:, PK_XT : PK_XT + KC * ROWS] = (
            xt.reshape(KC, 128, ROWS).transpose(1, 0, 2).reshape(128, KC * ROWS)
        )
        pack[0:ROWS, PK_ID : PK_ID + ROWS] = np.eye(ROWS, dtype=np.float32)
        pack[0:E, PK_CT : PK_CT + ROWS] = C[rs].T
        # c-broadcast row (expanded to all 128 partitions on-chip):
        # crow[0, 128e + 64j + b] = C[rs][b, e]
        crow = np.broadcast_to(C[rs].T[:, None, :], (E, 2, ROWS)).reshape(1, -1)
        in_maps.append(
            {
                "pack": pack.astype(mm_np),
                "crow": crow.astype(mm_np),
                "biasd": Bb,
                "w": W,
            }
        )
    return in_maps


def run(inputs, mode=MODE, trace=False):
    """Returns (output [512,512] fp32, BassKernelResults)."""
    from concourse.bass_utils import run_bass_kernel_spmd

    nc = _get_nc(mode)
    in_maps = _prep_in_maps(inputs, mode)
    res = run_bass_kernel_spmd(nc, in_maps, list(range(NCORES)), trace=trace)
    out = np.concatenate([r["out"] for r in res.results], axis=0)
    return out, res


def kernel(**inputs) -> np.ndarray:
    out, _ = run(inputs)
    return out
